# revision 1
# baseline (speedup 1.0000x reference)
"""DualGCN (two 2-layer GCN branches, concat) on 8 Trainium2 NeuronCores.

Math: gcn(x) = D^-1/2 (A+I) D^-1/2 (xW) + b (b asserted zero). With
dinv = deg^-1/2 folded node-wise:
  xt = dinv*x (host), h = xt @ W, z[row] = sum over in-edges (incl self-loop)
  of h[src_row]; internal layer emits relu(dinv^2 * z) (prescaled for the next
  layer), final layer emits relu(dinv * z).

Distribution: branch A (edge_index) on cores 0-3, branch C (edge_index_cross)
on cores 4-7; nodes relabeled into 128-row dst blocks with uniform in-degree
((deg, loA) two-level sort), blocks dealt round-robin to the 4 cores of the
branch. Layer-1 feature matmul is computed redundantly on every core (kills the
first all-gather); layer-2 matmul is sharded and its result exchanged at the
layer boundary.

Aggregation: per dst block, edge src-rows are gathered with gpsimd dma_gather
(int16 indices) from two overlapping 32768-row windows of the h buffer
(A=[0,32768), B=[17536,50304)) and accumulated into PSUM with identity-lhsT
matmuls; eviction fuses relu+scale on the scalar engine. Padding entries point
at dedicated zero rows. 4 SWDGE queues round-robin the gathers.
"""
import sys
sys.path.insert(0, "/opt/trn_rl_repo")
import numpy as np
import ml_dtypes

N = 50000
NP = 50176
D = 512
NBUF = 50304
PB = 17536
WIN = 32768
ZA = 0
ZB_ABS = 50240
ZB = ZB_ABS - PB
NBLK = 98
SPG = 8
TWO_NEFF = True   # layer boundary exchanged through the host (two NEFFs)


def _wrap_idx(flat_i16):
    S = len(flat_i16) // 16
    a = np.asarray(flat_i16, dtype=np.int16).reshape(S, 16).T
    return np.tile(a, (8, 1))


def group_sizes(n):
    out = []
    while n > 0:
        out.append(min(SPG, n))
        n -= out[-1]
    return out


def build_branch(edge_index):
    src = np.asarray(edge_index[0], dtype=np.int64)
    dst = np.asarray(edge_index[1], dtype=np.int64)
    loop = np.arange(N, dtype=np.int64)
    src = np.concatenate([src, loop])
    dst = np.concatenate([dst, loop])

    deg = np.bincount(dst, minlength=NP).astype(np.int64)
    dinv = np.zeros(NP, np.float64)
    nz = deg > 0
    dinv[nz] = 1.0 / np.sqrt(deg[nz].astype(np.float64))

    def rows_from_order(order):
        rows = np.empty(NP, np.int64)
        b = np.arange(392)
        base = 1 + ((b % 4) * NBLK + b // 4) * 128
        rows[order.reshape(392, 128)] = base[:, None] + np.arange(128)[None, :]
        return rows

    order0 = np.argsort(deg, kind="stable")
    rows0 = rows_from_order(order0)
    loA0 = np.bincount(dst[rows0[src] < PB], minlength=NP)
    order1 = np.lexsort((loA0, deg))
    rows = rows_from_order(order1)
    blocks = order1.reshape(392, 128)

    src_rows = rows[src]
    ordE = np.lexsort((src_rows, dst))
    s_dst = dst[ordE]
    s_sr = src_rows[ordE]
    starts = np.searchsorted(s_dst, np.arange(NP))
    mustA = np.bincount(dst[src_rows < PB], minlength=NP)
    canA = np.bincount(dst[src_rows < WIN], minlength=NP)

    cores = []
    for c in range(4):
        blks = {}
        for j in range(NBLK):
            nodes = blocks[j * 4 + c]
            blks[j] = dict(nodes=nodes, deg=deg[nodes], mA=mustA[nodes],
                           cA=canA[nodes], starts=starts[nodes])
        cores.append(dict(blocks=blks))
    return dict(cores=cores, rows=rows, dinv=dinv, deg=deg, s_sr=s_sr)


def equalize_structure(brA, brC):
    # Coordinate the A/B split point T_j across all 8 cores so the equalized
    # per-slot structure has minimal padding, then derive per-lane t.
    allc = brA["cores"] + brC["cores"]
    struct = []
    for j in range(NBLK):
        T = max(int(c["blocks"][j]["mA"].max()) for c in allc)
        sA = sB = 0
        for c in allc:
            blk = c["blocks"][j]
            t = np.clip(T, blk["mA"], blk["cA"])
            blk["t"] = t
            sA = max(sA, int(t.max()))
            sB = max(sB, int((blk["deg"] - t).max()))
        if sA + sB == 0:
            sA = 1
        struct.append((sA, sB))
    return struct


def build_core_tables(br, c, struct):
    core = br["cores"][c]
    s_sr = br["s_sr"]
    cols = []
    for j in range(NBLK):
        sA_j, sB_j = struct[j]
        blk = core["blocks"][j]
        t = blk["t"]; dg = blk["deg"]; st = blk["starts"]
        tabA = np.full((sA_j, 128), ZA, np.int64)
        for p in range(128):
            tp = int(t[p])
            if tp:
                tabA[:tp, p] = s_sr[st[p]:st[p] + tp]
        assert tabA.max() < WIN and tabA.min() >= 0
        tabB = np.full((sB_j, 128), ZB, np.int64)
        for p in range(128):
            nb = int(dg[p] - t[p])
            if nb:
                tabB[:nb, p] = s_sr[st[p] + t[p]:st[p] + dg[p]] - PB
        if sB_j:
            assert tabB.max() < WIN and tabB.min() >= 0
        ptr = 0
        for g in group_sizes(sA_j):
            cols.append(_wrap_idx(tabA[ptr:ptr + g].ravel()))
            ptr += g
        ptr = 0
        for g in group_sizes(sB_j):
            cols.append(_wrap_idx(tabB[ptr:ptr + g].ravel()))
            ptr += g
    return np.concatenate(cols, axis=1)


def _emit_agg(nc, tc, bass, mybir, struct, idxt, dvt, hsrc_win, pools, layer,
              x2, out, next_q, dep_inst=None):
    """Emit aggregation for one layer. hsrc_win(page)->AP of 32768-row window.
    dep_inst: instruction every gather must wait on (h buffer fully written) —
    Tile does not track DRAM-tile read-after-write for dma_gather sources."""
    from concourse.tile_rust import add_dep_helper
    Relu = mybir.ActivationFunctionType.Relu
    gpool, epool, zpp, ident = pools
    last_evict = [None]
    ci = [0]
    for j in range(NBLK):
        sA_j, sB_j = struct[j]
        total_mm = sA_j + sB_j
        pz = zpp.tile([128, D], mybir.dt.float32)
        n_mm = 0
        for page, cnt in (("A", sA_j), ("B", sB_j)):
            for gsz in group_sizes(cnt):
                g = gpool.tile([128, SPG, D], mybir.dt.bfloat16)
                c0 = ci[0]
                ci[0] += gsz * 8
                gi = nc.gpsimd.dma_gather(
                    g[:, :gsz, :], hsrc_win(page), idxt[:, c0:c0 + gsz * 8],
                    gsz * 128, gsz * 128, D, queue_num=next_q())
                if dep_inst is not None:
                    add_dep_helper(gi.ins, dep_inst,
                                   reason="gather waits for h buffer writes")
                for k in range(gsz):
                    nc.tensor.matmul(pz[:], ident[:], g[:, k, :],
                                     start=(n_mm == 0),
                                     stop=(n_mm == total_mm - 1))
                    n_mm += 1
        rs = slice(j * 128, (j + 1) * 128)
        if layer == 1:
            ev = epool.tile([128, D], mybir.dt.bfloat16, tag="evs")
            nc.scalar.activation(ev[:], pz[:], Relu, scale=dvt[:, j:j + 1])
            last_evict[0] = nc.sync.dma_start(out=x2[rs, :], in_=ev[:])
        else:
            evf = epool.tile([128, D], mybir.dt.float32, tag="evf")
            nc.scalar.activation(evf[:], pz[:], Relu,
                                 scale=dvt[:, NBLK + j:NBLK + j + 1])
            nc.sync.dma_start(out=out[rs, :], in_=evf[:])
    return last_evict[0]


def _mk_queue_fn():
    qn = [0]
    def next_q():
        qn[0] = (qn[0] + 1) % 4
        return qn[0]
    return next_q


def build_neff_a(struct, totc):
    """P1 (redundant full layer-1 matmul) + layer-1 aggregation + layer-2
    feature matmul. Outputs hs2 [12544, 512] bf16 (this core's h2 shard)."""
    import concourse.bass as bass
    import concourse.mybir as mybir
    import concourse.tile as tile
    from concourse import bacc
    from concourse.masks import make_identity

    nc = bacc.Bacc("TRN2", target_bir_lowering=False, debug=False,
                   num_swdge_queues=4)
    bf16, f32, i16 = mybir.dt.bfloat16, mybir.dt.float32, mybir.dt.int16
    Copy = mybir.ActivationFunctionType.Copy
    xT = nc.declare_dram_parameter("xT", [NP // 256, D, 256], bf16, isOutput=False)
    W1 = nc.declare_dram_parameter("W1", [D, D], bf16, isOutput=False)
    W2 = nc.declare_dram_parameter("W2", [D, D], bf16, isOutput=False)
    idx = nc.declare_dram_parameter("idx", [128, totc], i16, isOutput=False)
    dvec = nc.declare_dram_parameter("dvec", [128, 2 * NBLK], f32, isOutput=False)
    hs2 = nc.declare_dram_parameter("hs2", [NBLK * 128, D], bf16, isOutput=True)
    next_q = _mk_queue_fn()

    with tile.TileContext(nc) as tc:
        with (
            tc.tile_pool(name="dram", bufs=1, space="DRAM") as dpool,
            tc.tile_pool(name="const", bufs=1) as cpool,
            tc.tile_pool(name="xs", bufs=3) as xpool,
            tc.tile_pool(name="gt", bufs=3) as gpool,
            tc.tile_pool(name="ev", bufs=2) as epool,
            tc.tile_pool(name="hp", bufs=2, space="PSUM") as hpp,
            tc.tile_pool(name="zp", bufs=4, space="PSUM") as zpp,
        ):
            h1 = dpool.tile([NBUF, D], bf16)
            x2 = dpool.tile([NBLK * 128, D], bf16)

            ident = cpool.tile([128, 128], bf16)
            make_identity(nc, ident[:])
            w1t = cpool.tile([128, 4, D], bf16)
            nc.sync.dma_start(out=w1t[:], in_=W1[:].rearrange("(k c) n -> c k n", c=128))
            w2t = cpool.tile([128, 4, D], bf16)
            nc.sync.dma_start(out=w2t[:], in_=W2[:].rearrange("(k c) n -> c k n", c=128))
            idxt = cpool.tile([128, totc], i16)
            nc.sync.dma_start(out=idxt[:], in_=idx[:])
            dvt = cpool.tile([128, 2 * NBLK], f32)
            nc.sync.dma_start(out=dvt[:], in_=dvec[:])
            zt = cpool.tile([128, D], bf16)
            nc.gpsimd.memset(zt[:], 0.0)
            nc.sync.dma_start(out=h1[ZA:ZA + 1, :], in_=zt[:1, :])
            nc.sync.dma_start(out=h1[ZB_ABS:ZB_ABS + 1, :], in_=zt[:1, :])

            for gp in range(196):
                xt_t = xpool.tile([128, 4, 256], bf16, tag="xt")
                nc.sync.dma_start(out=xt_t[:],
                                  in_=xT[gp].rearrange("(k c) n -> c k n", c=128))
                ph = hpp.tile([128, 2, D], f32)
                for half in range(2):
                    for ck in range(4):
                        nc.tensor.matmul(
                            ph[:, half, :], xt_t[:, ck, bass.ts(half, 128)],
                            w1t[:, ck, :], start=(ck == 0), stop=(ck == 3))
                ev = epool.tile([128, 2 * D], bf16, tag="evb")
                nc.scalar.activation(ev[:], ph[:].rearrange("p a b -> p (a b)"), Copy)
                wlast = nc.sync.dma_start(
                    out=h1[1 + gp * 256:1 + (gp + 1) * 256, :].rearrange(
                        "(a p) b -> p a b", p=128),
                    in_=ev[:].rearrange("p (a b) -> p a b", b=D))

            from concourse.tile_rust import add_dep_helper
            def win1(page):
                return h1[0:WIN, :] if page == "A" else h1[PB:PB + WIN, :]
            x2last = _emit_agg(nc, tc, bass, mybir, struct, idxt, dvt, win1,
                               (gpool, epool, zpp, ident), 1, x2, None, next_q,
                               dep_inst=wlast.ins)

            for gp in range(49):
                x2t = xpool.tile([128, 4, 256], bf16, tag="x2t")
                for ck in range(4):
                    ti = nc.sync.dma_start(
                        out=x2t[:, ck, :],
                        in_=x2[gp * 256:(gp + 1) * 256, ck * 128:(ck + 1) * 128],
                        transpose=True)
                    add_dep_helper(ti.ins, x2last.ins,
                                   reason="transpose waits for x2 writes")
                ph = hpp.tile([128, 2, D], f32)
                for half in range(2):
                    for ck in range(4):
                        nc.tensor.matmul(
                            ph[:, half, :], x2t[:, ck, bass.ts(half, 128)],
                            w2t[:, ck, :], start=(ck == 0), stop=(ck == 3))
                ev = epool.tile([128, 2 * D], bf16, tag="evb")
                nc.scalar.activation(ev[:], ph[:].rearrange("p a b -> p (a b)"), Copy)
                nc.sync.dma_start(
                    out=hs2[gp * 256:(gp + 1) * 256, :].rearrange(
                        "(a p) b -> p a b", p=128),
                    in_=ev[:].rearrange("p (a b) -> p a b", b=D))
    nc.finalize()
    return nc


def build_neff_b(struct, totc):
    """Layer-2 aggregation from a host-assembled full h2 buffer."""
    import concourse.bass as bass
    import concourse.mybir as mybir
    import concourse.tile as tile
    from concourse import bacc
    from concourse.masks import make_identity

    nc = bacc.Bacc("TRN2", target_bir_lowering=False, debug=False,
                   num_swdge_queues=4)
    bf16, f32, i16 = mybir.dt.bfloat16, mybir.dt.float32, mybir.dt.int16
    h2 = nc.declare_dram_parameter("h2", [NBUF, D], bf16, isOutput=False)
    idx = nc.declare_dram_parameter("idx", [128, totc], i16, isOutput=False)
    dvec = nc.declare_dram_parameter("dvec", [128, 2 * NBLK], f32, isOutput=False)
    out = nc.declare_dram_parameter("out", [NBLK * 128, D], f32, isOutput=True)
    next_q = _mk_queue_fn()

    with tile.TileContext(nc) as tc:
        with (
            tc.tile_pool(name="const", bufs=1) as cpool,
            tc.tile_pool(name="gt", bufs=10) as gpool,
            tc.tile_pool(name="ev", bufs=4) as epool,
            tc.tile_pool(name="zp", bufs=7, space="PSUM") as zpp,
        ):
            ident = cpool.tile([128, 128], bf16)
            make_identity(nc, ident[:])
            idxt = cpool.tile([128, totc], i16)
            nc.sync.dma_start(out=idxt[:], in_=idx[:])
            dvt = cpool.tile([128, 2 * NBLK], f32)
            nc.sync.dma_start(out=dvt[:], in_=dvec[:])

            def win2(page):
                return h2[0:WIN, :] if page == "A" else h2[PB:PB + WIN, :]
            _emit_agg(nc, tc, bass, mybir, struct, idxt, dvt, win2,
                      (gpool, epool, zpp, ident), 2, None, out, next_q)
    nc.finalize()
    return nc


def build_single_neff(struct, totc):
    """Single-NEFF variant with on-device AllGather at the layer boundary."""
    import concourse.bass as bass
    import concourse.mybir as mybir
    import concourse.tile as tile
    from concourse import bacc
    from concourse.masks import make_identity

    nc = bacc.Bacc("TRN2", target_bir_lowering=False, debug=False,
                   num_swdge_queues=4)
    bf16, f32, i16 = mybir.dt.bfloat16, mybir.dt.float32, mybir.dt.int16
    Copy = mybir.ActivationFunctionType.Copy
    xT = nc.declare_dram_parameter("xT", [NP // 256, D, 256], bf16, isOutput=False)
    W1 = nc.declare_dram_parameter("W1", [D, D], bf16, isOutput=False)
    W2 = nc.declare_dram_parameter("W2", [D, D], bf16, isOutput=False)
    idx = nc.declare_dram_parameter("idx", [128, totc], i16, isOutput=False)
    dvec = nc.declare_dram_parameter("dvec", [128, 2 * NBLK], f32, isOutput=False)
    out = nc.declare_dram_parameter("out", [NBLK * 128, D], f32, isOutput=True)
    next_q = _mk_queue_fn()

    with tile.TileContext(nc) as tc:
        with (
            tc.tile_pool(name="dram", bufs=1, space="DRAM") as dpool,
            tc.tile_pool(name="const", bufs=1) as cpool,
            tc.tile_pool(name="xs", bufs=3) as xpool,
            tc.tile_pool(name="gt", bufs=3) as gpool,
            tc.tile_pool(name="ev", bufs=2) as epool,
            tc.tile_pool(name="hp", bufs=2, space="PSUM") as hpp,
            tc.tile_pool(name="zp", bufs=4, space="PSUM") as zpp,
        ):
            h1 = dpool.tile([NBUF, D], bf16)
            h2 = dpool.tile([NBUF, D], bf16)
            hs2 = dpool.tile([NBLK * 128, D], bf16)
            x2 = dpool.tile([NBLK * 128, D], bf16)

            ident = cpool.tile([128, 128], bf16)
            make_identity(nc, ident[:])
            w1t = cpool.tile([128, 4, D], bf16)
            nc.sync.dma_start(out=w1t[:], in_=W1[:].rearrange("(k c) n -> c k n", c=128))
            w2t = cpool.tile([128, 4, D], bf16)
            nc.sync.dma_start(out=w2t[:], in_=W2[:].rearrange("(k c) n -> c k n", c=128))
            idxt = cpool.tile([128, totc], i16)
            nc.sync.dma_start(out=idxt[:], in_=idx[:])
            dvt = cpool.tile([128, 2 * NBLK], f32)
            nc.sync.dma_start(out=dvt[:], in_=dvec[:])
            zt = cpool.tile([128, D], bf16)
            nc.gpsimd.memset(zt[:], 0.0)
            for hb in (h1, h2):
                nc.sync.dma_start(out=hb[ZA:ZA + 1, :], in_=zt[:1, :])
                nc.sync.dma_start(out=hb[ZB_ABS:ZB_ABS + 1, :], in_=zt[:1, :])

            for gp in range(196):
                xt_t = xpool.tile([128, 4, 256], bf16, tag="xt")
                nc.sync.dma_start(out=xt_t[:],
                                  in_=xT[gp].rearrange("(k c) n -> c k n", c=128))
                ph = hpp.tile([128, 2, D], f32)
                for half in range(2):
                    for ck in range(4):
                        nc.tensor.matmul(
                            ph[:, half, :], xt_t[:, ck, bass.ts(half, 128)],
                            w1t[:, ck, :], start=(ck == 0), stop=(ck == 3))
                ev = epool.tile([128, 2 * D], bf16, tag="evb")
                nc.scalar.activation(ev[:], ph[:].rearrange("p a b -> p (a b)"), Copy)
                nc.sync.dma_start(
                    out=h1[1 + gp * 256:1 + (gp + 1) * 256, :].rearrange(
                        "(a p) b -> p a b", p=128),
                    in_=ev[:].rearrange("p (a b) -> p a b", b=D))

            def win1(page):
                return h1[0:WIN, :] if page == "A" else h1[PB:PB + WIN, :]
            _emit_agg(nc, tc, bass, mybir, struct, idxt, dvt, win1,
                      (gpool, epool, zpp, ident), 1, x2, None, next_q)

            for gp in range(49):
                x2t = xpool.tile([128, 4, 256], bf16, tag="x2t")
                for ck in range(4):
                    nc.sync.dma_start(
                        out=x2t[:, ck, :],
                        in_=x2[gp * 256:(gp + 1) * 256, ck * 128:(ck + 1) * 128],
                        transpose=True)
                ph = hpp.tile([128, 2, D], f32)
                for half in range(2):
                    for ck in range(4):
                        nc.tensor.matmul(
                            ph[:, half, :], x2t[:, ck, bass.ts(half, 128)],
                            w2t[:, ck, :], start=(ck == 0), stop=(ck == 3))
                ev = epool.tile([128, 2 * D], bf16, tag="evb")
                nc.scalar.activation(ev[:], ph[:].rearrange("p a b -> p (a b)"), Copy)
                nc.sync.dma_start(
                    out=hs2[gp * 256:(gp + 1) * 256, :].rearrange(
                        "(a p) b -> p a b", p=128),
                    in_=ev[:].rearrange("p (a b) -> p a b", b=D))

            nc.gpsimd.collective_compute(
                "AllGather", mybir.AluOpType.bypass,
                replica_groups=[[0, 1, 2, 3], [4, 5, 6, 7]],
                ins=[hs2[:].opt()],
                outs=[h2[1:1 + 4 * NBLK * 128, :].opt()])

            def win2(page):
                return h2[0:WIN, :] if page == "A" else h2[PB:PB + WIN, :]
            _emit_agg(nc, tc, bass, mybir, struct, idxt, dvt, win2,
                      (gpool, epool, zpp, ident), 2, None, out, next_q)
    nc.finalize()
    return nc


def _prep(x, edge_index, edge_index_cross, W1, W2, Wc1, Wc2):
    x = np.asarray(x, np.float32)
    brA = build_branch(np.asarray(edge_index))
    brC = build_branch(np.asarray(edge_index_cross))
    struct = equalize_structure(brA, brC)
    in_maps = []
    for c in range(8):
        br = brA if c < 4 else brC
        idx = build_core_tables(br, c % 4, struct)
        rows = br["rows"]; dinv = br["dinv"]; deg = br["deg"]
        xt = np.zeros((NP, D), np.float32)
        pos = rows - 1
        xt[pos[:N]] = x * dinv[:N, None].astype(np.float32)
        xTf = np.ascontiguousarray(xt.T).astype(ml_dtypes.bfloat16)
        xT = np.ascontiguousarray(
            xTf.reshape(D, NP // 256, 256).transpose(1, 0, 2))
        dv = np.zeros((128, 2 * NBLK), np.float32)
        for j in range(NBLK):
            nodes = br["cores"][c % 4]["blocks"][j]["nodes"]
            dgn = deg[nodes]
            with np.errstate(divide="ignore"):
                dv[:, j] = np.where(dgn > 0, 1.0 / dgn, 0.0)
            dv[:, NBLK + j] = dinv[nodes]
        Wa = np.asarray(W1 if c < 4 else Wc1, np.float32).astype(ml_dtypes.bfloat16)
        Wb = np.asarray(W2 if c < 4 else Wc2, np.float32).astype(ml_dtypes.bfloat16)
        in_maps.append(dict(xT=xT, W1=np.ascontiguousarray(Wa),
                            W2=np.ascontiguousarray(Wb), idx=idx, dvec=dv))
    totc = in_maps[0]["idx"].shape[1]
    return brA, brC, struct, totc, in_maps


_CACHE = {}


def kernel_merged(x, edge_index, edge_index_cross, W1, b1, W2, b2,
           Wc1, bc1, Wc2, bc2, _collect_exec_ns=None, _trace=False):
    from concourse import bass_utils
    bass_utils.upload_artifacts = lambda t: "local://" + t
    from concourse.bass_utils import run_bass_kernel_spmd

    for b in (b1, b2, bc1, bc2):
        assert not np.any(np.asarray(b)), "nonzero bias not supported"

    brA, brC, struct, totc, in_maps = _prep(
        x, edge_index, edge_index_cross, W1, W2, Wc1, Wc2)

    exec_ns = 0
    if TWO_NEFF:
        key = ("A", totc, tuple(struct))
        if key not in _CACHE:
            _CACHE[key] = build_neff_a(struct, totc)
        ncA = _CACHE[key]
        resA = run_bass_kernel_spmd(ncA, in_maps, core_ids=list(range(8)),
                                    trace=_trace)
        if resA.exec_time_ns:
            exec_ns += resA.exec_time_ns
        # assemble full h2 per branch on host
        maps_b = []
        for half in range(2):
            h2 = np.zeros((NBUF, D), ml_dtypes.bfloat16)
            h2[1:1 + 4 * NBLK * 128] = np.concatenate(
                [resA.results[half * 4 + c]["hs2"] for c in range(4)], axis=0)
            for c in range(4):
                maps_b.append(dict(
                    h2=h2, idx=in_maps[half * 4 + c]["idx"],
                    dvec=in_maps[half * 4 + c]["dvec"]))
        maps_b = maps_b[:4] + maps_b[4:]
        keyb = ("B", totc, tuple(struct))
        if keyb not in _CACHE:
            _CACHE[keyb] = build_neff_b(struct, totc)
        ncB = _CACHE[keyb]
        resB = run_bass_kernel_spmd(ncB, maps_b, core_ids=list(range(8)),
                                    trace=_trace)
        if resB.exec_time_ns:
            exec_ns += resB.exec_time_ns
        results = resB.results
    else:
        key = ("S", totc, tuple(struct))
        if key not in _CACHE:
            _CACHE[key] = build_single_neff(struct, totc)
        res = run_bass_kernel_spmd(_CACHE[key], in_maps,
                                   core_ids=list(range(8)), trace=_trace)
        if res.exec_time_ns:
            exec_ns += res.exec_time_ns
        results = res.results

    if _collect_exec_ns is not None:
        _collect_exec_ns.append(exec_ns)

    full = np.zeros((N, 2 * D), np.float32)
    for half, br in ((0, brA), (1, brC)):
        stack = np.concatenate(
            [results[half * 4 + c]["out"] for c in range(4)], axis=0)
        pos = br["rows"][:N] - 1
        full[:, half * D:(half + 1) * D] = stack[pos]
    return full


def build_mm(totc_unused=None):
    """Sharded feature matmul: hsh[12544,512]bf16 = xTs-blocked @ W."""
    import concourse.bass as bass
    import concourse.mybir as mybir
    import concourse.tile as tile
    from concourse import bacc
    nc = bacc.Bacc("TRN2", target_bir_lowering=False, debug=False)
    bf16, f32 = mybir.dt.bfloat16, mybir.dt.float32
    Copy = mybir.ActivationFunctionType.Copy
    xTs = nc.declare_dram_parameter("xTs", [49, D, 256], bf16, isOutput=False)
    W = nc.declare_dram_parameter("W", [D, D], bf16, isOutput=False)
    hsh = nc.declare_dram_parameter("hsh", [NBLK * 128, D], bf16, isOutput=True)
    with tile.TileContext(nc) as tc:
        with (
            tc.tile_pool(name="const", bufs=1) as cpool,
            tc.tile_pool(name="xs", bufs=4) as xpool,
            tc.tile_pool(name="ev", bufs=3) as epool,
            tc.tile_pool(name="hp", bufs=3, space="PSUM") as hpp,
        ):
            wt = cpool.tile([128, 4, D], bf16)
            nc.sync.dma_start(out=wt[:], in_=W[:].rearrange("(k c) n -> c k n", c=128))
            for gp in range(49):
                xt_t = xpool.tile([128, 4, 256], bf16, tag="xt")
                nc.sync.dma_start(out=xt_t[:],
                                  in_=xTs[gp].rearrange("(k c) n -> c k n", c=128))
                ph = hpp.tile([128, 2, D], f32)
                for half in range(2):
                    for ck in range(4):
                        nc.tensor.matmul(
                            ph[:, half, :], xt_t[:, ck, bass.ts(half, 128)],
                            wt[:, ck, :], start=(ck == 0), stop=(ck == 3))
                ev = epool.tile([128, 2 * D], bf16, tag="evb")
                nc.scalar.activation(ev[:], ph[:].rearrange("p a b -> p (a b)"), Copy)
                nc.sync.dma_start(
                    out=hsh[gp * 256:(gp + 1) * 256, :].rearrange(
                        "(a p) b -> p a b", p=128),
                    in_=ev[:].rearrange("p (a b) -> p a b", b=D))
    nc.finalize()
    return nc


def build_agg(struct, totc, layer):
    """Aggregation layer from a full h parameter. layer 1 -> bf16 prescaled
    x2; layer 2 -> f32 final out."""
    import concourse.bass as bass
    import concourse.mybir as mybir
    import concourse.tile as tile
    from concourse import bacc
    from concourse.masks import make_identity
    nc = bacc.Bacc("TRN2", target_bir_lowering=False, debug=False,
                   num_swdge_queues=4)
    bf16, f32, i16 = mybir.dt.bfloat16, mybir.dt.float32, mybir.dt.int16
    h = nc.declare_dram_parameter("h", [NBUF, D], bf16, isOutput=False)
    idx = nc.declare_dram_parameter("idx", [128, totc], i16, isOutput=False)
    dvec = nc.declare_dram_parameter("dvec", [128, 2 * NBLK], f32, isOutput=False)
    odt = bf16 if layer == 1 else f32
    out = nc.declare_dram_parameter("out", [NBLK * 128, D], odt, isOutput=True)
    next_q = _mk_queue_fn()
    with tile.TileContext(nc) as tc:
        with (
            tc.tile_pool(name="const", bufs=1) as cpool,
            tc.tile_pool(name="gt", bufs=10) as gpool,
            tc.tile_pool(name="ev", bufs=4) as epool,
            tc.tile_pool(name="zp", bufs=7, space="PSUM") as zpp,
        ):
            ident = cpool.tile([128, 128], bf16)
            make_identity(nc, ident[:])
            idxt = cpool.tile([128, totc], i16)
            nc.sync.dma_start(out=idxt[:], in_=idx[:])
            dvt = cpool.tile([128, 2 * NBLK], f32)
            nc.sync.dma_start(out=dvt[:], in_=dvec[:])

            def win(page):
                return h[0:WIN, :] if page == "A" else h[PB:PB + WIN, :]
            _emit_agg(nc, tc, bass, mybir, struct, idxt, dvt, win,
                      (gpool, epool, zpp, ident), layer,
                      out if layer == 1 else None,
                      out if layer == 2 else None, next_q)
    nc.finalize()
    return nc


def _blocked_T(xrows):
    """[12544, 512] -> blocked transposed [49, 512, 256] bf16."""
    a = np.ascontiguousarray(np.asarray(xrows, dtype=ml_dtypes.bfloat16).T)
    return np.ascontiguousarray(a.reshape(D, 49, 256).transpose(1, 0, 2))


def kernel_four(x, edge_index, edge_index_cross, W1, b1, W2, b2,
                Wc1, bc1, Wc2, bc2, _collect_exec_ns=None, _trace=False):
    from concourse import bass_utils
    bass_utils.upload_artifacts = lambda t: "local://" + t
    from concourse.bass_utils import run_bass_kernel_spmd

    for b in (b1, b2, bc1, bc2):
        assert not np.any(np.asarray(b)), "nonzero bias not supported"
    brA, brC, struct, totc, in_maps = _prep(
        x, edge_index, edge_index_cross, W1, W2, Wc1, Wc2)

    if "M" not in _CACHE:
        _CACHE["M"] = build_mm()
    if ("G1", totc) not in _CACHE:
        _CACHE[("G1", totc)] = build_agg(struct, totc, 1)
    if ("G2", totc) not in _CACHE:
        _CACHE[("G2", totc)] = build_agg(struct, totc, 2)
    ncM, ncG1, ncG2 = _CACHE["M"], _CACHE[("G1", totc)], _CACHE[("G2", totc)]
    exec_ns = 0

    def runit(nc, maps):
        nonlocal exec_ns
        r = run_bass_kernel_spmd(nc, maps, core_ids=list(range(8)), trace=_trace)
        if r.exec_time_ns:
            exec_ns += r.exec_time_ns
        return r.results

    # per-core x~ shard (branch row order), blocked-transposed
    xsh = []
    xf = np.asarray(x, np.float32)
    for c in range(8):
        br = brA if c < 4 else brC
        rows = br["rows"]; dinv = br["dinv"]
        xt = np.zeros((NP, D), np.float32)
        pos = rows - 1
        xt[pos[:N]] = xf * dinv[:N, None].astype(np.float32)
        lo = (c % 4) * NBLK * 128
        xsh.append(xt[lo:lo + NBLK * 128])

    def mm_pass(shards, Wmats):
        maps = [dict(xTs=_blocked_T(shards[c]), W=Wmats[c]) for c in range(8)]
        res = runit(ncM, maps)
        h = []
        for half in range(2):
            hf = np.zeros((NBUF, D), ml_dtypes.bfloat16)
            hf[1:1 + 4 * NBLK * 128] = np.concatenate(
                [res[half * 4 + c]["hsh"] for c in range(4)], axis=0)
            h.append(hf)
        return h

    Wa1 = [in_maps[c]["W1"] for c in range(8)]
    Wa2 = [in_maps[c]["W2"] for c in range(8)]
    h1 = mm_pass(xsh, Wa1)
    maps_g = [dict(h=h1[c // 4], idx=in_maps[c]["idx"],
                   dvec=in_maps[c]["dvec"]) for c in range(8)]
    resG1 = runit(ncG1, maps_g)
    x2 = [resG1[c]["out"] for c in range(8)]
    h2 = mm_pass(x2, Wa2)
    maps_g2 = [dict(h=h2[c // 4], idx=in_maps[c]["idx"],
                    dvec=in_maps[c]["dvec"]) for c in range(8)]
    resG2 = runit(ncG2, maps_g2)

    if _collect_exec_ns is not None:
        _collect_exec_ns.append(exec_ns)
    full = np.zeros((N, 2 * D), np.float32)
    for half, br in ((0, brA), (1, brC)):
        stack = np.concatenate(
            [resG2[half * 4 + c]["out"] for c in range(4)], axis=0)
        pos = br["rows"][:N] - 1
        full[:, half * D:(half + 1) * D] = stack[pos]
    return full


def kernel(**kw):
    return kernel_four(**kw)



# revision 2
# speedup vs baseline: 1.0095x; 1.0095x over previous
"""DualGCN (two 2-layer GCN branches, concat) on 8 Trainium2 NeuronCores.

Math: gcn(x) = D^-1/2 (A+I) D^-1/2 (xW) + b (b asserted zero). With
dinv = deg^-1/2 folded node-wise:
  xt = dinv*x (host), h = xt @ W, z[row] = sum over in-edges (incl self-loop)
  of h[src_row]; internal layer emits relu(dinv^2 * z) (prescaled for the next
  layer), final layer emits relu(dinv * z).

Distribution: branch A (edge_index) on cores 0-3, branch C (edge_index_cross)
on cores 4-7; nodes relabeled into 128-row dst blocks with uniform in-degree
((deg, loA) two-level sort), blocks dealt round-robin to the 4 cores of the
branch. Layer-1 feature matmul is computed redundantly on every core (kills the
first all-gather); layer-2 matmul is sharded and its result exchanged at the
layer boundary.

Aggregation: per dst block, edge src-rows are gathered with gpsimd dma_gather
(int16 indices) from two overlapping 32768-row windows of the h buffer
(A=[0,32768), B=[17536,50304)) and accumulated into PSUM with identity-lhsT
matmuls; eviction fuses relu+scale on the scalar engine. Padding entries point
at dedicated zero rows. 4 SWDGE queues round-robin the gathers.
"""
import sys
sys.path.insert(0, "/opt/trn_rl_repo")
import numpy as np
import ml_dtypes

N = 50000
NP = 50176
D = 512
NBUF = 50304
PB = 17536
WIN = 32768
ZA = 0
ZB_ABS = 50240
ZB = ZB_ABS - PB
NBLK = 98
SPG = 8
TWO_NEFF = True   # layer boundary exchanged through the host (two NEFFs)


def _wrap_idx(flat_i16):
    S = len(flat_i16) // 16
    a = np.asarray(flat_i16, dtype=np.int16).reshape(S, 16).T
    return np.tile(a, (8, 1))


def group_sizes(n):
    out = []
    while n > 0:
        out.append(min(SPG, n))
        n -= out[-1]
    return out


def build_branch(edge_index):
    src = np.asarray(edge_index[0], dtype=np.int64)
    dst = np.asarray(edge_index[1], dtype=np.int64)
    loop = np.arange(N, dtype=np.int64)
    src = np.concatenate([src, loop])
    dst = np.concatenate([dst, loop])

    deg = np.bincount(dst, minlength=NP).astype(np.int64)
    dinv = np.zeros(NP, np.float64)
    nz = deg > 0
    dinv[nz] = 1.0 / np.sqrt(deg[nz].astype(np.float64))

    def rows_from_order(order):
        rows = np.empty(NP, np.int64)
        b = np.arange(392)
        base = 1 + ((b % 4) * NBLK + b // 4) * 128
        rows[order.reshape(392, 128)] = base[:, None] + np.arange(128)[None, :]
        return rows

    order0 = np.argsort(deg, kind="stable")
    rows0 = rows_from_order(order0)
    loA0 = np.bincount(dst[rows0[src] < PB], minlength=NP)
    order1 = np.lexsort((loA0, deg))
    rows = rows_from_order(order1)
    blocks = order1.reshape(392, 128)

    src_rows = rows[src]
    ordE = np.lexsort((src_rows, dst))
    s_dst = dst[ordE]
    s_sr = src_rows[ordE]
    starts = np.searchsorted(s_dst, np.arange(NP))
    mustA = np.bincount(dst[src_rows < PB], minlength=NP)
    canA = np.bincount(dst[src_rows < WIN], minlength=NP)

    cores = []
    for c in range(4):
        blks = {}
        for j in range(NBLK):
            nodes = blocks[j * 4 + c]
            blks[j] = dict(nodes=nodes, deg=deg[nodes], mA=mustA[nodes],
                           cA=canA[nodes], starts=starts[nodes])
        cores.append(dict(blocks=blks))
    return dict(cores=cores, rows=rows, dinv=dinv, deg=deg, s_sr=s_sr)


def equalize_structure(brA, brC):
    # Coordinate the A/B split point T_j across all 8 cores so the equalized
    # per-slot structure has minimal padding, then derive per-lane t.
    allc = brA["cores"] + brC["cores"]
    struct = []
    for j in range(NBLK):
        T = max(int(c["blocks"][j]["mA"].max()) for c in allc)
        sA = sB = 0
        for c in allc:
            blk = c["blocks"][j]
            t = np.clip(T, blk["mA"], blk["cA"])
            blk["t"] = t
            sA = max(sA, int(t.max()))
            sB = max(sB, int((blk["deg"] - t).max()))
        if sA + sB == 0:
            sA = 1
        struct.append((sA, sB))
    return struct


def build_core_tables(br, c, struct):
    core = br["cores"][c]
    s_sr = br["s_sr"]
    cols = []
    for j in range(NBLK):
        sA_j, sB_j = struct[j]
        blk = core["blocks"][j]
        t = blk["t"]; dg = blk["deg"]; st = blk["starts"]
        tabA = np.full((sA_j, 128), ZA, np.int64)
        for p in range(128):
            tp = int(t[p])
            if tp:
                tabA[:tp, p] = s_sr[st[p]:st[p] + tp]
        assert tabA.max() < WIN and tabA.min() >= 0
        tabB = np.full((sB_j, 128), ZB, np.int64)
        for p in range(128):
            nb = int(dg[p] - t[p])
            if nb:
                tabB[:nb, p] = s_sr[st[p] + t[p]:st[p] + dg[p]] - PB
        if sB_j:
            assert tabB.max() < WIN and tabB.min() >= 0
        ptr = 0
        for g in group_sizes(sA_j):
            cols.append(_wrap_idx(tabA[ptr:ptr + g].ravel()))
            ptr += g
        ptr = 0
        for g in group_sizes(sB_j):
            cols.append(_wrap_idx(tabB[ptr:ptr + g].ravel()))
            ptr += g
    return np.concatenate(cols, axis=1)


def _emit_agg(nc, tc, bass, mybir, struct, idxt, dvt, hsrc_win, pools, layer,
              x2, out, next_q, dep_inst=None):
    """Emit aggregation for one layer. hsrc_win(page)->AP of 32768-row window.
    dep_inst: instruction every gather must wait on (h buffer fully written) —
    Tile does not track DRAM-tile read-after-write for dma_gather sources."""
    from concourse.tile_rust import add_dep_helper
    Relu = mybir.ActivationFunctionType.Relu
    gpool, epool, zpp, ident = pools
    last_evict = [None]
    ci = [0]
    for j in range(NBLK):
        sA_j, sB_j = struct[j]
        total_mm = sA_j + sB_j
        pz = zpp.tile([128, D], mybir.dt.float32)
        n_mm = 0
        for page, cnt in (("A", sA_j), ("B", sB_j)):
            for gsz in group_sizes(cnt):
                g = gpool.tile([128, SPG, D], mybir.dt.bfloat16)
                c0 = ci[0]
                ci[0] += gsz * 8
                gi = nc.gpsimd.dma_gather(
                    g[:, :gsz, :], hsrc_win(page), idxt[:, c0:c0 + gsz * 8],
                    gsz * 128, gsz * 128, D, queue_num=next_q())
                if dep_inst is not None:
                    add_dep_helper(gi.ins, dep_inst,
                                   reason="gather waits for h buffer writes")
                for k in range(gsz):
                    nc.tensor.matmul(pz[:], ident[:], g[:, k, :],
                                     start=(n_mm == 0),
                                     stop=(n_mm == total_mm - 1))
                    n_mm += 1
        rs = slice(j * 128, (j + 1) * 128)
        if layer == 1:
            ev = epool.tile([128, D], mybir.dt.bfloat16, tag="evs")
            nc.scalar.activation(ev[:], pz[:], Relu, scale=dvt[:, j:j + 1])
            last_evict[0] = nc.sync.dma_start(out=x2[rs, :], in_=ev[:])
        else:
            evf = epool.tile([128, D], mybir.dt.float32, tag="evf")
            nc.scalar.activation(evf[:], pz[:], Relu,
                                 scale=dvt[:, NBLK + j:NBLK + j + 1])
            nc.sync.dma_start(out=out[rs, :], in_=evf[:])
    return last_evict[0]


def _mk_queue_fn():
    qn = [0]
    def next_q():
        qn[0] = (qn[0] + 1) % 4
        return qn[0]
    return next_q


def build_neff_a(struct, totc):
    """P1 (redundant full layer-1 matmul) + layer-1 aggregation + layer-2
    feature matmul. Outputs hs2 [12544, 512] bf16 (this core's h2 shard)."""
    import concourse.bass as bass
    import concourse.mybir as mybir
    import concourse.tile as tile
    from concourse import bacc
    from concourse.masks import make_identity

    nc = bacc.Bacc("TRN2", target_bir_lowering=False, debug=False,
                   num_swdge_queues=4)
    bf16, f32, i16 = mybir.dt.bfloat16, mybir.dt.float32, mybir.dt.int16
    Copy = mybir.ActivationFunctionType.Copy
    xT = nc.declare_dram_parameter("xT", [NP // 256, D, 256], bf16, isOutput=False)
    W1 = nc.declare_dram_parameter("W1", [D, D], bf16, isOutput=False)
    W2 = nc.declare_dram_parameter("W2", [D, D], bf16, isOutput=False)
    idx = nc.declare_dram_parameter("idx", [128, totc], i16, isOutput=False)
    dvec = nc.declare_dram_parameter("dvec", [128, 2 * NBLK], f32, isOutput=False)
    hs2 = nc.declare_dram_parameter("hs2", [NBLK * 128, D], bf16, isOutput=True)
    next_q = _mk_queue_fn()

    with tile.TileContext(nc) as tc:
        with (
            tc.tile_pool(name="dram", bufs=1, space="DRAM") as dpool,
            tc.tile_pool(name="const", bufs=1) as cpool,
            tc.tile_pool(name="xs", bufs=3) as xpool,
            tc.tile_pool(name="gt", bufs=3) as gpool,
            tc.tile_pool(name="ev", bufs=2) as epool,
            tc.tile_pool(name="hp", bufs=2, space="PSUM") as hpp,
            tc.tile_pool(name="zp", bufs=4, space="PSUM") as zpp,
        ):
            h1 = dpool.tile([NBUF, D], bf16)
            x2 = dpool.tile([NBLK * 128, D], bf16)

            ident = cpool.tile([128, 128], bf16)
            make_identity(nc, ident[:])
            w1t = cpool.tile([128, 4, D], bf16)
            nc.sync.dma_start(out=w1t[:], in_=W1[:].rearrange("(k c) n -> c k n", c=128))
            w2t = cpool.tile([128, 4, D], bf16)
            nc.sync.dma_start(out=w2t[:], in_=W2[:].rearrange("(k c) n -> c k n", c=128))
            idxt = cpool.tile([128, totc], i16)
            nc.sync.dma_start(out=idxt[:], in_=idx[:])
            dvt = cpool.tile([128, 2 * NBLK], f32)
            nc.sync.dma_start(out=dvt[:], in_=dvec[:])
            zt = cpool.tile([128, D], bf16)
            nc.gpsimd.memset(zt[:], 0.0)
            nc.sync.dma_start(out=h1[ZA:ZA + 1, :], in_=zt[:1, :])
            nc.sync.dma_start(out=h1[ZB_ABS:ZB_ABS + 1, :], in_=zt[:1, :])

            for gp in range(196):
                xt_t = xpool.tile([128, 4, 256], bf16, tag="xt")
                nc.sync.dma_start(out=xt_t[:],
                                  in_=xT[gp].rearrange("(k c) n -> c k n", c=128))
                ph = hpp.tile([128, 2, D], f32)
                for half in range(2):
                    for ck in range(4):
                        nc.tensor.matmul(
                            ph[:, half, :], xt_t[:, ck, bass.ts(half, 128)],
                            w1t[:, ck, :], start=(ck == 0), stop=(ck == 3))
                ev = epool.tile([128, 2 * D], bf16, tag="evb")
                nc.scalar.activation(ev[:], ph[:].rearrange("p a b -> p (a b)"), Copy)
                wlast = nc.sync.dma_start(
                    out=h1[1 + gp * 256:1 + (gp + 1) * 256, :].rearrange(
                        "(a p) b -> p a b", p=128),
                    in_=ev[:].rearrange("p (a b) -> p a b", b=D))

            from concourse.tile_rust import add_dep_helper
            def win1(page):
                return h1[0:WIN, :] if page == "A" else h1[PB:PB + WIN, :]
            x2last = _emit_agg(nc, tc, bass, mybir, struct, idxt, dvt, win1,
                               (gpool, epool, zpp, ident), 1, x2, None, next_q,
                               dep_inst=wlast.ins)

            for gp in range(49):
                x2t = xpool.tile([128, 4, 256], bf16, tag="x2t")
                for ck in range(4):
                    ti = nc.sync.dma_start(
                        out=x2t[:, ck, :],
                        in_=x2[gp * 256:(gp + 1) * 256, ck * 128:(ck + 1) * 128],
                        transpose=True)
                    add_dep_helper(ti.ins, x2last.ins,
                                   reason="transpose waits for x2 writes")
                ph = hpp.tile([128, 2, D], f32)
                for half in range(2):
                    for ck in range(4):
                        nc.tensor.matmul(
                            ph[:, half, :], x2t[:, ck, bass.ts(half, 128)],
                            w2t[:, ck, :], start=(ck == 0), stop=(ck == 3))
                ev = epool.tile([128, 2 * D], bf16, tag="evb")
                nc.scalar.activation(ev[:], ph[:].rearrange("p a b -> p (a b)"), Copy)
                nc.sync.dma_start(
                    out=hs2[gp * 256:(gp + 1) * 256, :].rearrange(
                        "(a p) b -> p a b", p=128),
                    in_=ev[:].rearrange("p (a b) -> p a b", b=D))
    nc.finalize()
    return nc


def build_neff_b(struct, totc):
    """Layer-2 aggregation from a host-assembled full h2 buffer."""
    import concourse.bass as bass
    import concourse.mybir as mybir
    import concourse.tile as tile
    from concourse import bacc
    from concourse.masks import make_identity

    nc = bacc.Bacc("TRN2", target_bir_lowering=False, debug=False,
                   num_swdge_queues=4)
    bf16, f32, i16 = mybir.dt.bfloat16, mybir.dt.float32, mybir.dt.int16
    h2 = nc.declare_dram_parameter("h2", [NBUF, D], bf16, isOutput=False)
    idx = nc.declare_dram_parameter("idx", [128, totc], i16, isOutput=False)
    dvec = nc.declare_dram_parameter("dvec", [128, 2 * NBLK], f32, isOutput=False)
    out = nc.declare_dram_parameter("out", [NBLK * 128, D], f32, isOutput=True)
    next_q = _mk_queue_fn()

    with tile.TileContext(nc) as tc:
        with (
            tc.tile_pool(name="const", bufs=1) as cpool,
            tc.tile_pool(name="gt", bufs=10) as gpool,
            tc.tile_pool(name="ev", bufs=4) as epool,
            tc.tile_pool(name="zp", bufs=7, space="PSUM") as zpp,
        ):
            ident = cpool.tile([128, 128], bf16)
            make_identity(nc, ident[:])
            idxt = cpool.tile([128, totc], i16)
            nc.sync.dma_start(out=idxt[:], in_=idx[:])
            dvt = cpool.tile([128, 2 * NBLK], f32)
            nc.sync.dma_start(out=dvt[:], in_=dvec[:])

            def win2(page):
                return h2[0:WIN, :] if page == "A" else h2[PB:PB + WIN, :]
            _emit_agg(nc, tc, bass, mybir, struct, idxt, dvt, win2,
                      (gpool, epool, zpp, ident), 2, None, out, next_q)
    nc.finalize()
    return nc


def build_single_neff(struct, totc):
    """Single-NEFF variant with on-device AllGather at the layer boundary."""
    import concourse.bass as bass
    import concourse.mybir as mybir
    import concourse.tile as tile
    from concourse import bacc
    from concourse.masks import make_identity

    nc = bacc.Bacc("TRN2", target_bir_lowering=False, debug=False,
                   num_swdge_queues=4)
    bf16, f32, i16 = mybir.dt.bfloat16, mybir.dt.float32, mybir.dt.int16
    Copy = mybir.ActivationFunctionType.Copy
    xT = nc.declare_dram_parameter("xT", [NP // 256, D, 256], bf16, isOutput=False)
    W1 = nc.declare_dram_parameter("W1", [D, D], bf16, isOutput=False)
    W2 = nc.declare_dram_parameter("W2", [D, D], bf16, isOutput=False)
    idx = nc.declare_dram_parameter("idx", [128, totc], i16, isOutput=False)
    dvec = nc.declare_dram_parameter("dvec", [128, 2 * NBLK], f32, isOutput=False)
    out = nc.declare_dram_parameter("out", [NBLK * 128, D], f32, isOutput=True)
    next_q = _mk_queue_fn()

    with tile.TileContext(nc) as tc:
        with (
            tc.tile_pool(name="dram", bufs=1, space="DRAM") as dpool,
            tc.tile_pool(name="const", bufs=1) as cpool,
            tc.tile_pool(name="xs", bufs=3) as xpool,
            tc.tile_pool(name="gt", bufs=3) as gpool,
            tc.tile_pool(name="ev", bufs=2) as epool,
            tc.tile_pool(name="hp", bufs=2, space="PSUM") as hpp,
            tc.tile_pool(name="zp", bufs=4, space="PSUM") as zpp,
        ):
            h1 = dpool.tile([NBUF, D], bf16)
            h2 = dpool.tile([NBUF, D], bf16)
            hs2 = dpool.tile([NBLK * 128, D], bf16)
            x2 = dpool.tile([NBLK * 128, D], bf16)

            ident = cpool.tile([128, 128], bf16)
            make_identity(nc, ident[:])
            w1t = cpool.tile([128, 4, D], bf16)
            nc.sync.dma_start(out=w1t[:], in_=W1[:].rearrange("(k c) n -> c k n", c=128))
            w2t = cpool.tile([128, 4, D], bf16)
            nc.sync.dma_start(out=w2t[:], in_=W2[:].rearrange("(k c) n -> c k n", c=128))
            idxt = cpool.tile([128, totc], i16)
            nc.sync.dma_start(out=idxt[:], in_=idx[:])
            dvt = cpool.tile([128, 2 * NBLK], f32)
            nc.sync.dma_start(out=dvt[:], in_=dvec[:])
            zt = cpool.tile([128, D], bf16)
            nc.gpsimd.memset(zt[:], 0.0)
            for hb in (h1, h2):
                nc.sync.dma_start(out=hb[ZA:ZA + 1, :], in_=zt[:1, :])
                nc.sync.dma_start(out=hb[ZB_ABS:ZB_ABS + 1, :], in_=zt[:1, :])

            for gp in range(196):
                xt_t = xpool.tile([128, 4, 256], bf16, tag="xt")
                nc.sync.dma_start(out=xt_t[:],
                                  in_=xT[gp].rearrange("(k c) n -> c k n", c=128))
                ph = hpp.tile([128, 2, D], f32)
                for half in range(2):
                    for ck in range(4):
                        nc.tensor.matmul(
                            ph[:, half, :], xt_t[:, ck, bass.ts(half, 128)],
                            w1t[:, ck, :], start=(ck == 0), stop=(ck == 3))
                ev = epool.tile([128, 2 * D], bf16, tag="evb")
                nc.scalar.activation(ev[:], ph[:].rearrange("p a b -> p (a b)"), Copy)
                nc.sync.dma_start(
                    out=h1[1 + gp * 256:1 + (gp + 1) * 256, :].rearrange(
                        "(a p) b -> p a b", p=128),
                    in_=ev[:].rearrange("p (a b) -> p a b", b=D))

            def win1(page):
                return h1[0:WIN, :] if page == "A" else h1[PB:PB + WIN, :]
            _emit_agg(nc, tc, bass, mybir, struct, idxt, dvt, win1,
                      (gpool, epool, zpp, ident), 1, x2, None, next_q)

            for gp in range(49):
                x2t = xpool.tile([128, 4, 256], bf16, tag="x2t")
                for ck in range(4):
                    nc.sync.dma_start(
                        out=x2t[:, ck, :],
                        in_=x2[gp * 256:(gp + 1) * 256, ck * 128:(ck + 1) * 128],
                        transpose=True)
                ph = hpp.tile([128, 2, D], f32)
                for half in range(2):
                    for ck in range(4):
                        nc.tensor.matmul(
                            ph[:, half, :], x2t[:, ck, bass.ts(half, 128)],
                            w2t[:, ck, :], start=(ck == 0), stop=(ck == 3))
                ev = epool.tile([128, 2 * D], bf16, tag="evb")
                nc.scalar.activation(ev[:], ph[:].rearrange("p a b -> p (a b)"), Copy)
                nc.sync.dma_start(
                    out=hs2[gp * 256:(gp + 1) * 256, :].rearrange(
                        "(a p) b -> p a b", p=128),
                    in_=ev[:].rearrange("p (a b) -> p a b", b=D))

            nc.gpsimd.collective_compute(
                "AllGather", mybir.AluOpType.bypass,
                replica_groups=[[0, 1, 2, 3], [4, 5, 6, 7]],
                ins=[hs2[:].opt()],
                outs=[h2[1:1 + 4 * NBLK * 128, :].opt()])

            def win2(page):
                return h2[0:WIN, :] if page == "A" else h2[PB:PB + WIN, :]
            _emit_agg(nc, tc, bass, mybir, struct, idxt, dvt, win2,
                      (gpool, epool, zpp, ident), 2, None, out, next_q)
    nc.finalize()
    return nc


def _prep(x, edge_index, edge_index_cross, W1, W2, Wc1, Wc2):
    x = np.asarray(x, np.float32)
    brA = build_branch(np.asarray(edge_index))
    brC = build_branch(np.asarray(edge_index_cross))
    struct = equalize_structure(brA, brC)
    in_maps = []
    for c in range(8):
        br = brA if c < 4 else brC
        idx = build_core_tables(br, c % 4, struct)
        rows = br["rows"]; dinv = br["dinv"]; deg = br["deg"]
        xt = np.zeros((NP, D), np.float32)
        pos = rows - 1
        xt[pos[:N]] = x * dinv[:N, None].astype(np.float32)
        xTf = np.ascontiguousarray(xt.T).astype(ml_dtypes.bfloat16)
        xT = np.ascontiguousarray(
            xTf.reshape(D, NP // 256, 256).transpose(1, 0, 2))
        dv = np.zeros((128, 2 * NBLK), np.float32)
        for j in range(NBLK):
            nodes = br["cores"][c % 4]["blocks"][j]["nodes"]
            dgn = deg[nodes]
            with np.errstate(divide="ignore"):
                dv[:, j] = np.where(dgn > 0, 1.0 / dgn, 0.0)
            dv[:, NBLK + j] = dinv[nodes]
        Wa = np.asarray(W1 if c < 4 else Wc1, np.float32).astype(ml_dtypes.bfloat16)
        Wb = np.asarray(W2 if c < 4 else Wc2, np.float32).astype(ml_dtypes.bfloat16)
        in_maps.append(dict(xT=xT, W1=np.ascontiguousarray(Wa),
                            W2=np.ascontiguousarray(Wb), idx=idx, dvec=dv))
    totc = in_maps[0]["idx"].shape[1]
    return brA, brC, struct, totc, in_maps


_CACHE = {}


def kernel_merged(x, edge_index, edge_index_cross, W1, b1, W2, b2,
           Wc1, bc1, Wc2, bc2, _collect_exec_ns=None, _trace=False):
    from concourse import bass_utils
    bass_utils.upload_artifacts = lambda t: "local://" + t
    from concourse.bass_utils import run_bass_kernel_spmd

    for b in (b1, b2, bc1, bc2):
        assert not np.any(np.asarray(b)), "nonzero bias not supported"

    brA, brC, struct, totc, in_maps = _prep(
        x, edge_index, edge_index_cross, W1, W2, Wc1, Wc2)

    exec_ns = 0
    if TWO_NEFF:
        key = ("A", totc, tuple(struct))
        if key not in _CACHE:
            _CACHE[key] = build_neff_a(struct, totc)
        ncA = _CACHE[key]
        resA = run_bass_kernel_spmd(ncA, in_maps, core_ids=list(range(8)),
                                    trace=_trace)
        if resA.exec_time_ns:
            exec_ns += resA.exec_time_ns
        # assemble full h2 per branch on host
        maps_b = []
        for half in range(2):
            h2 = np.zeros((NBUF, D), ml_dtypes.bfloat16)
            h2[1:1 + 4 * NBLK * 128] = np.concatenate(
                [resA.results[half * 4 + c]["hs2"] for c in range(4)], axis=0)
            for c in range(4):
                maps_b.append(dict(
                    h2=h2, idx=in_maps[half * 4 + c]["idx"],
                    dvec=in_maps[half * 4 + c]["dvec"]))
        maps_b = maps_b[:4] + maps_b[4:]
        keyb = ("B", totc, tuple(struct))
        if keyb not in _CACHE:
            _CACHE[keyb] = build_neff_b(struct, totc)
        ncB = _CACHE[keyb]
        resB = run_bass_kernel_spmd(ncB, maps_b, core_ids=list(range(8)),
                                    trace=_trace)
        if resB.exec_time_ns:
            exec_ns += resB.exec_time_ns
        results = resB.results
    else:
        key = ("S", totc, tuple(struct))
        if key not in _CACHE:
            _CACHE[key] = build_single_neff(struct, totc)
        res = run_bass_kernel_spmd(_CACHE[key], in_maps,
                                   core_ids=list(range(8)), trace=_trace)
        if res.exec_time_ns:
            exec_ns += res.exec_time_ns
        results = res.results

    if _collect_exec_ns is not None:
        _collect_exec_ns.append(exec_ns)

    full = np.zeros((N, 2 * D), np.float32)
    for half, br in ((0, brA), (1, brC)):
        stack = np.concatenate(
            [results[half * 4 + c]["out"] for c in range(4)], axis=0)
        pos = br["rows"][:N] - 1
        full[:, half * D:(half + 1) * D] = stack[pos]
    return full


def build_mm(totc_unused=None):
    """Sharded feature matmul: hsh[12544,512]bf16 = xTs-blocked @ W."""
    import concourse.bass as bass
    import concourse.mybir as mybir
    import concourse.tile as tile
    from concourse import bacc
    nc = bacc.Bacc("TRN2", target_bir_lowering=False, debug=False)
    bf16, f32 = mybir.dt.bfloat16, mybir.dt.float32
    Copy = mybir.ActivationFunctionType.Copy
    xTs = nc.declare_dram_parameter("xTs", [49, D, 256], bf16, isOutput=False)
    W = nc.declare_dram_parameter("W", [D, D], bf16, isOutput=False)
    hsh = nc.declare_dram_parameter("hsh", [NBLK * 128, D], bf16, isOutput=True)
    with tile.TileContext(nc) as tc:
        with (
            tc.tile_pool(name="const", bufs=1) as cpool,
            tc.tile_pool(name="xs", bufs=4) as xpool,
            tc.tile_pool(name="ev", bufs=3) as epool,
            tc.tile_pool(name="hp", bufs=3, space="PSUM") as hpp,
        ):
            wt = cpool.tile([128, 4, D], bf16)
            nc.sync.dma_start(out=wt[:], in_=W[:].rearrange("(k c) n -> c k n", c=128))
            for gp in range(49):
                xt_t = xpool.tile([128, 4, 256], bf16, tag="xt")
                nc.sync.dma_start(out=xt_t[:],
                                  in_=xTs[gp].rearrange("(k c) n -> c k n", c=128))
                ph = hpp.tile([128, 2, D], f32)
                for half in range(2):
                    for ck in range(4):
                        nc.tensor.matmul(
                            ph[:, half, :], xt_t[:, ck, bass.ts(half, 128)],
                            wt[:, ck, :], start=(ck == 0), stop=(ck == 3))
                ev = epool.tile([128, 2 * D], bf16, tag="evb")
                nc.scalar.activation(ev[:], ph[:].rearrange("p a b -> p (a b)"), Copy)
                nc.sync.dma_start(
                    out=hsh[gp * 256:(gp + 1) * 256, :].rearrange(
                        "(a p) b -> p a b", p=128),
                    in_=ev[:].rearrange("p (a b) -> p a b", b=D))
    nc.finalize()
    return nc


def build_agg(struct, totc, layer):
    """Aggregation layer from a full h parameter. layer 1 -> bf16 prescaled
    x2; layer 2 -> f32 final out."""
    import concourse.bass as bass
    import concourse.mybir as mybir
    import concourse.tile as tile
    from concourse import bacc
    from concourse.masks import make_identity
    nc = bacc.Bacc("TRN2", target_bir_lowering=False, debug=False,
                   num_swdge_queues=4)
    bf16, f32, i16 = mybir.dt.bfloat16, mybir.dt.float32, mybir.dt.int16
    h = nc.declare_dram_parameter("h", [NBUF, D], bf16, isOutput=False)
    idx = nc.declare_dram_parameter("idx", [128, totc], i16, isOutput=False)
    dvec = nc.declare_dram_parameter("dvec", [128, 2 * NBLK], f32, isOutput=False)
    odt = bf16 if layer == 1 else f32
    out = nc.declare_dram_parameter("out", [NBLK * 128, D], odt, isOutput=True)
    next_q = _mk_queue_fn()
    with tile.TileContext(nc) as tc:
        with (
            tc.tile_pool(name="const", bufs=1) as cpool,
            tc.tile_pool(name="gt", bufs=10) as gpool,
            tc.tile_pool(name="ev", bufs=4) as epool,
            tc.tile_pool(name="zp", bufs=7, space="PSUM") as zpp,
        ):
            ident = cpool.tile([128, 128], bf16)
            make_identity(nc, ident[:])
            idxt = cpool.tile([128, totc], i16)
            nc.sync.dma_start(out=idxt[:], in_=idx[:])
            dvt = cpool.tile([128, 2 * NBLK], f32)
            nc.sync.dma_start(out=dvt[:], in_=dvec[:])

            def win(page):
                return h[0:WIN, :] if page == "A" else h[PB:PB + WIN, :]
            _emit_agg(nc, tc, bass, mybir, struct, idxt, dvt, win,
                      (gpool, epool, zpp, ident), layer,
                      out if layer == 1 else None,
                      out if layer == 2 else None, next_q)
    nc.finalize()
    return nc


def _blocked_T(xrows):
    """[12544, 512] -> blocked transposed [49, 512, 256] bf16."""
    a = np.ascontiguousarray(np.asarray(xrows, dtype=ml_dtypes.bfloat16).T)
    return np.ascontiguousarray(a.reshape(D, 49, 256).transpose(1, 0, 2))


def kernel_four(x, edge_index, edge_index_cross, W1, b1, W2, b2,
                Wc1, bc1, Wc2, bc2, _collect_exec_ns=None, _trace=False):
    from concourse import bass_utils
    bass_utils.upload_artifacts = lambda t: "local://" + t
    from concourse.bass_utils import run_bass_kernel_spmd

    for b in (b1, b2, bc1, bc2):
        assert not np.any(np.asarray(b)), "nonzero bias not supported"
    brA, brC, struct, totc, in_maps = _prep(
        x, edge_index, edge_index_cross, W1, W2, Wc1, Wc2)

    if "M" not in _CACHE:
        _CACHE["M"] = build_mm()
    if ("G1", totc) not in _CACHE:
        _CACHE[("G1", totc)] = build_agg(struct, totc, 1)
    if ("G2", totc) not in _CACHE:
        _CACHE[("G2", totc)] = build_agg(struct, totc, 2)
    ncM, ncG1, ncG2 = _CACHE["M"], _CACHE[("G1", totc)], _CACHE[("G2", totc)]
    exec_ns = 0

    import os as _os
    def runit(nc, maps):
        nonlocal exec_ns
        r = run_bass_kernel_spmd(nc, maps, core_ids=list(range(8)), trace=_trace)
        if r.exec_time_ns:
            exec_ns += r.exec_time_ns
        if _os.environ.get("DBG_EXEC"):
            print("RUN exec_ns:", r.exec_time_ns)
        return r.results

    # per-core x~ shard (branch row order), blocked-transposed
    xsh = []
    xf = np.asarray(x, np.float32)
    for c in range(8):
        br = brA if c < 4 else brC
        rows = br["rows"]; dinv = br["dinv"]
        xt = np.zeros((NP, D), np.float32)
        pos = rows - 1
        xt[pos[:N]] = xf * dinv[:N, None].astype(np.float32)
        lo = (c % 4) * NBLK * 128
        xsh.append(xt[lo:lo + NBLK * 128])

    def mm_pass(shards, Wmats):
        maps = [dict(xTs=_blocked_T(shards[c]), W=Wmats[c]) for c in range(8)]
        res = runit(ncM, maps)
        h = []
        for half in range(2):
            hf = np.zeros((NBUF, D), ml_dtypes.bfloat16)
            hf[1:1 + 4 * NBLK * 128] = np.concatenate(
                [res[half * 4 + c]["hsh"] for c in range(4)], axis=0)
            h.append(hf)
        return h

    Wa1 = [in_maps[c]["W1"] for c in range(8)]
    Wa2 = [in_maps[c]["W2"] for c in range(8)]
    h1 = mm_pass(xsh, Wa1)
    maps_g = [dict(h=h1[c // 4], idx=in_maps[c]["idx"],
                   dvec=in_maps[c]["dvec"]) for c in range(8)]
    resG1 = runit(ncG1, maps_g)
    x2 = [resG1[c]["out"] for c in range(8)]
    h2 = mm_pass(x2, Wa2)
    maps_g2 = [dict(h=h2[c // 4], idx=in_maps[c]["idx"],
                    dvec=in_maps[c]["dvec"]) for c in range(8)]
    resG2 = runit(ncG2, maps_g2)

    if _collect_exec_ns is not None:
        _collect_exec_ns.append(exec_ns)
    full = np.zeros((N, 2 * D), np.float32)
    for half, br in ((0, brA), (1, brC)):
        stack = np.concatenate(
            [resG2[half * 4 + c]["out"] for c in range(4)], axis=0)
        pos = br["rows"][:N] - 1
        full[:, half * D:(half + 1) * D] = stack[pos]
    return full


def kernel(**kw):
    return kernel_four(**kw)

